# revision 5
# baseline (speedup 1.0000x reference)
"""Trainium2 Bass kernel for nn_GapDecoder.

Computes gaps[i,j] = proj[i] + proj[j] + b2 where
proj = relu(x @ W1 + b1) @ w2, x: [8192, 512] f32.

Strategy (8 NeuronCores, block-partitioned, collective-free, quantized):
  gaps is symmetric, so only ~half of the 8x8 grid of [1024,1024] blocks
  is materialized on device (host mirrors transposes); output is uint8
  with a per-block scale (max quant err ~0.4% « the 2e-2 budget); x is
  read in bf16. ~8.6MB HBM traffic per core vs 40MB f32 baseline.

  Core m holds chunks Lm = (m, m+1, m+2, m+4) (mod 8); identical SPMD
  graph per core, blocks in LOCAL indices:
    (0,0) diag triangle [DVE]  (0,1) full [Pool]  (0,2) full [DVE]
    (1,3) full [ACT]           (0,3) triangle of (m, m+4) [Pool]
  The (0,3) triangle union with the partner core's transposed triangle
  covers that block; triangle strip g spans cols [g*128, 1024).

  Quantization per block (rows p, cols q):
    f = 126.9/(Gp + Gq + |b2|), G = max|proj| over the chunk, computed
    on the partition-broadcast bcol so a [128,1] free-dim reduce is
    globally valid. u8 = round(f*(proj_i+proj_j+b2) + 128) in [1.1,255).

  Scheduling notes (from perfetto traces):
  - read DMAs are issued before const DMAs, and chunk 0 is fetched as
    two stripe-major halves so the PE starts ~7us earlier
  - emissions are queued as pending thunks and flushed interleaved with
    later chunks' compute ops, so a block's 8 strips don't head-of-line
    block the next chunk's relu/cast on the same engine queue
  - DVE emits via tmpq = f*bcol + c2 then 1-ALU-op adds (740ns/1024
    cols vs 970 for 2-op); ACT uses its fused scale+bias form; Pool
    only ever runs 2-op tensor_scalar (its 1-op lowering is ~10x slow)
  - relu runs on DVE for the first two chunks (ACT is cast-loaded)
  - full-block writes are split into two [128,4096] DMAs so the last
    block's write overlaps its second half's strips
"""

import sys

sys.path.insert(0, "/opt/trn_rl_repo")

import numpy as np

N, D, H = 8192, 512, 32
NCORES = 8
CHUNK = 1024
STRIPE = 512
KCH = D // 128  # 4
QSCALE = 126.9

LOCAL_OFFS = (0, 1, 2, 4)
COMPUTE_ORDER = (0, 1, 3, 2)
# (row_local, col_local, kind, fslot)
BLOCKS = (
    (0, 0, "tri", 0),  # diag     DVE   tri slot 0
    (0, 1, "full", 1),  # d1      Pool  odd row 0
    (0, 2, "full", 2),  # d2      DVE   odd row 1
    (1, 3, "full", 4),  # d3      ACT   odd row 2
    (0, 3, "tri", 3),  # d4 half  Pool  tri slot 1
)
ORow = {1: 0, 2: 1, 4: 2}
TRI_OFF = [g * CHUNK - 64 * g * (g - 1) for g in range(9)]
TRI_W = TRI_OFF[8]  # 4608

_state = {}
LAST_RESULTS = None


def _build():
    from collections import deque
    from concourse import bacc, tile, mybir

    f32 = mybir.dt.float32
    bf16 = mybir.dt.bfloat16
    u8 = mybir.dt.uint8
    A = mybir.AluOpType
    AF = mybir.ActivationFunctionType
    AX = mybir.AxisListType

    nc = bacc.Bacc(
        "TRN2", target_bir_lowering=False, debug=False, num_devices=NCORES
    )

    # x packed stripe-major per chunk: col = half*2048 + k*512 + j
    xT_d = nc.dram_tensor("xT4", [128, 4 * KCH * CHUNK], bf16, kind="ExternalInput")
    w1_d = nc.dram_tensor("W1b", [D, H], bf16, kind="ExternalInput")
    w2b_d = nc.dram_tensor("w2b", [H, 128], bf16, kind="ExternalInput")
    # f32 consts packed: col0 |b2|, col1 b2, col2 rows0-31 b1
    fp_d = nc.dram_tensor("fpack", [128, 3], f32, kind="ExternalInput")
    od_d = nc.dram_tensor("odd", [3 * 128, 8 * CHUNK], u8, kind="ExternalOutput")
    tr_d = nc.dram_tensor("outtr", [2 * 128, TRI_W], u8, kind="ExternalOutput")
    fv_d = nc.dram_tensor("fv", [1, 8], f32, kind="ExternalOutput")

    with tile.TileContext(nc) as tc:
        with (
            tc.tile_pool(name="const", bufs=1) as cpool,
            tc.tile_pool(name="work", bufs=4) as wpool,
            tc.tile_pool(name="psA", bufs=2, space="PSUM") as psA,
            tc.tile_pool(name="psB", bufs=2, space="PSUM") as psB,
            tc.tile_pool(name="psC", bufs=2, space="PSUM") as psC,
        ):
            # ---- read DMAs first: chunk0 in stripe halves, then 1,3,2 ----
            xks = {}
            rd = []
            for loc in range(4):
                xks[loc] = cpool.tile([128, KCH * CHUNK], bf16, name=f"xk{loc}")
            c00 = 0
            nc.sync.dma_start(xks[0][:, 0:2048], xT_d.ap()[:, 0:2048])
            nc.sync.dma_start(xks[0][:, 2048:4096], xT_d.ap()[:, 2048:4096])
            for loc in (1, 3, 2):
                c0 = loc * KCH * CHUNK
                nc.sync.dma_start(
                    xks[loc][:], xT_d.ap()[:, c0 : c0 + KCH * CHUNK]
                )

            # ---- consts ----
            w1_sb = cpool.tile([128, KCH, H], bf16)
            nc.sync.dma_start(
                w1_sb[:], w1_d.ap().rearrange("(k p) h -> p k h", p=128)
            )
            w2b_sb = cpool.tile([H, 128], bf16)
            nc.sync.dma_start(w2b_sb[:], w2b_d.ap())
            w2_sb = w2b_sb[:, 0:1]
            fp_sb = cpool.tile([128, 3], f32)
            nc.sync.dma_start(fp_sb[:], fp_d.ap())
            babs_sb = fp_sb[:, 0:1]
            brep_sb = fp_sb[:, 1:2]
            b1_sb = fp_sb[0:H, 2:3]

            # per-chunk persistent state
            bcol = [cpool.tile([128, CHUNK], bf16, name=f"bcol{i}") for i in range(4)]
            bcol3b = cpool.tile([128, CHUNK], bf16)
            projcol = [cpool.tile([128, 8], f32, name=f"pjc{i}") for i in range(2)]
            pmall = cpool.tile([128, 4], f32)
            gsum = cpool.tile([128, 5], f32)
            rr = cpool.tile([128, 5], f32)
            fsc = cpool.tile([128, 5], f32)
            c2 = cpool.tile([128, 5], f32)
            tks = [cpool.tile([128, 8], f32, name=f"tk{i}") for i in range(5)]
            tmpq = {0: cpool.tile([128, CHUNK], f32, name="tmpq0"),
                    2: cpool.tile([128, CHUNK], f32, name="tmpq2")}
            fvec = cpool.tile([1, 8], f32)
            nc.vector.memset(fvec[:], 1.0)
            ot = {1: cpool.tile([128, 8 * CHUNK], u8, name="ot1"),
                  2: cpool.tile([128, 8 * CHUNK], u8, name="ot2"),
                  3: cpool.tile([128, 8 * CHUNK], u8, name="ot3")}
            tri = {0: cpool.tile([128, TRI_W], u8, name="tri0"),
                   4: cpool.tile([128, TRI_W], u8, name="tri4")}

            pend = {"dve": deque(), "act": deque()}

            def flush(eng, n=10**9):
                q = pend[eng]
                while q and n > 0:
                    q.popleft()()
                    n -= 1

            def chain_group(slots, in0_ap, in1_ap):
                lo, hi = min(slots), max(slots) + 1
                nc.vector.scalar_tensor_tensor(
                    gsum[:, lo:hi], in0_ap, babs_sb, in1_ap,
                    op0=A.add, op1=A.add,
                )
                nc.vector.reciprocal(rr[:, lo:hi], gsum[:, lo:hi])
                nc.vector.tensor_scalar_mul(fsc[:, lo:hi], rr[:, lo:hi], QSCALE)
                nc.vector.tensor_scalar(
                    c2[:, lo:hi], fsc[:, lo:hi], brep_sb, 128.0,
                    op0=A.mult, op1=A.add,
                )
                nc.vector.tensor_copy(fvec[0:1, lo:hi], fsc[0:1, lo:hi])

            def emit(k):
                lp, lq, kind, fs = BLOCKS[k]
                fk = fsc[:, fs : fs + 1]
                if k in (0, 2):  # DVE: tmpq + 1-op strips
                    nc.vector.tensor_scalar_mul(tks[fs][:], projcol[lp][:], fk)
                    nc.vector.tensor_scalar(
                        tmpq[k][:], bcol[lq][:], fk, c2[:, fs : fs + 1],
                        op0=A.mult, op1=A.add,
                    )
                    if kind == "tri":
                        def strip(g):
                            w = CHUNK - 128 * g
                            nc.vector.tensor_scalar_add(
                                tri[0][:, TRI_OFF[g] : TRI_OFF[g] + w],
                                tmpq[0][:, g * 128 :],
                                tks[fs][:, g : g + 1],
                            )
                        for g in range(8):
                            pend["dve"].append(lambda g=g: strip(g))
                        pend["dve"].append(
                            lambda: nc.sync.dma_start(
                                tr_d.ap()[0:128, :], tri[0][:]
                            )
                        )
                    else:
                        b = ORow[fs]
                        def strip(g):
                            nc.vector.tensor_scalar_add(
                                ot[2][:, g * CHUNK : (g + 1) * CHUNK],
                                tmpq[2][:],
                                tks[fs][:, g : g + 1],
                            )
                        for half in range(2):
                            for g in range(4 * half, 4 * half + 4):
                                pend["dve"].append(lambda g=g: strip(g))
                            pend["dve"].append(
                                lambda b=b, half=half: nc.sync.dma_start(
                                    od_d.ap()[
                                        b * 128 : (b + 1) * 128,
                                        half * 4096 : (half + 1) * 4096,
                                    ],
                                    ot[2][:, half * 4096 : (half + 1) * 4096],
                                )
                            )
                elif k == 3:  # ACT fused strips
                    fs4 = fs
                    nc.vector.tensor_scalar(
                        tks[fs][:], projcol[lp][:], fk, c2[:, fs : fs + 1],
                        op0=A.mult, op1=A.add,
                    )
                    b = ORow[fs]
                    def strip(g):
                        nc.scalar.activation(
                            ot[3][:, g * CHUNK : (g + 1) * CHUNK],
                            bcol[3][:],
                            AF.Identity,
                            bias=tks[fs4][:, g : g + 1],
                            scale=fk,
                        )
                    for half in range(2):
                        for g in range(4 * half, 4 * half + 4):
                            pend["act"].append(lambda g=g: strip(g))
                        pend["act"].append(
                            lambda b=b, half=half: nc.sync.dma_start(
                                od_d.ap()[
                                    b * 128 : (b + 1) * 128,
                                    half * 4096 : (half + 1) * 4096,
                                ],
                                ot[3][:, half * 4096 : (half + 1) * 4096],
                            )
                        )
                elif k == 1:  # Pool 2-op, immediate
                    nc.vector.tensor_scalar(
                        tks[fs][:], projcol[lp][:], fk, c2[:, fs : fs + 1],
                        op0=A.mult, op1=A.add,
                    )
                    b = ORow[fs]
                    for half in range(2):
                        for g in range(4 * half, 4 * half + 4):
                            nc.gpsimd.tensor_scalar(
                                ot[1][:, g * CHUNK : (g + 1) * CHUNK],
                                bcol[1][:],
                                fk,
                                tks[fs][:, g : g + 1],
                                op0=A.mult,
                                op1=A.add,
                            )
                        nc.sync.dma_start(
                            od_d.ap()[
                                b * 128 : (b + 1) * 128,
                                half * 4096 : (half + 1) * 4096,
                            ],
                            ot[1][:, half * 4096 : (half + 1) * 4096],
                        )
                else:  # k == 4: d4h triangle, Pool 2-op from private copy
                    nc.vector.tensor_scalar(
                        tks[fs][:], projcol[lp][:], fk, c2[:, fs : fs + 1],
                        op0=A.mult, op1=A.add,
                    )
                    for g in range(8):
                        w = CHUNK - 128 * g
                        nc.gpsimd.tensor_scalar(
                            tri[4][:, TRI_OFF[g] : TRI_OFF[g] + w],
                            bcol3b[:, g * 128 :],
                            fk,
                            tks[fs][:, g : g + 1],
                            op0=A.mult,
                            op1=A.add,
                        )
                    nc.sync.dma_start(tr_d.ap()[128:256, :], tri[4][:])

            pcs = {}
            for li, loc in enumerate(COMPUTE_ORDER):
                xk = xks[loc]
                if loc < 2:
                    pcs[loc] = psC.tile([128, 8], f32, tag="pc", name=f"pc{loc}")
                for half in range(2):
                    seqT_ps = psA.tile([H, STRIPE], f32, tag="seqT")
                    for kk in range(KCH):
                        nc.tensor.matmul(
                            seqT_ps[:],
                            w1_sb[:, kk, :],
                            xk[:, half * 2048 + kk * STRIPE : half * 2048 + (kk + 1) * STRIPE],
                            start=(kk == 0),
                            stop=(kk == KCH - 1),
                        )
                    seqT_sb = wpool.tile([H, STRIPE], bf16, tag="seqT_sb")
                    if li < 2:
                        # relu on DVE while ACT warms up / casts
                        nc.vector.tensor_scalar(
                            seqT_sb[:], seqT_ps[:], b1_sb, 0.0,
                            op0=A.add, op1=A.max,
                        )
                    else:
                        nc.scalar.activation(
                            seqT_sb[:], seqT_ps[:], AF.Relu, bias=b1_sb, scale=1.0
                        )
                    bc_ps = psB.tile([128, STRIPE], f32, tag="bc")
                    nc.tensor.matmul(bc_ps[:], w2b_sb[:], seqT_sb[:])
                    nc.scalar.activation(
                        bcol[loc][:, half * STRIPE : (half + 1) * STRIPE],
                        bc_ps[:],
                        AF.Copy,
                    )
                    if loc < 2:
                        for c in range(4):
                            g = half * 4 + c
                            nc.tensor.matmul(
                                pcs[loc][:, g : g + 1],
                                seqT_sb[:, c * 128 : (c + 1) * 128],
                                w2_sb,
                                start=True,
                                stop=True,
                            )
                    flush("dve", 2)
                    flush("act", 2)
                if loc < 2:
                    nc.vector.tensor_copy(projcol[loc][:], pcs[loc][:])
                nc.vector.reduce_max(
                    pmall[:, loc : loc + 1],
                    bcol[loc][:],
                    axis=AX.X,
                    apply_absolute_value=True,
                )
                if loc == 3:
                    nc.vector.tensor_copy(bcol3b[:], bcol[3][:])

                if loc == 1:
                    chain_group(
                        (0, 1),
                        pmall[:, 0:1].broadcast_to([128, 2]),
                        pmall[:, 0:2],
                    )
                    emit(0)
                    emit(1)
                elif loc == 3:
                    chain_group(
                        (3, 4),
                        pmall[:, 0:2],
                        pmall[:, 3:4].broadcast_to([128, 2]),
                    )
                    emit(4)
                    emit(3)
                elif loc == 2:
                    chain_group((2,), pmall[:, 0:1], pmall[:, 2:3])
                    emit(2)

            flush("dve")
            flush("act")
            nc.sync.dma_start(fv_d.ap(), fvec[:])

    nc.compile()
    return nc


def _dequant(arr_u8, f):
    return (arr_u8.astype(np.float32) - 128.0) * (1.0 / f)


def _assemble(results):
    out = np.empty((N, N), dtype=np.float32)
    ii = (np.arange(CHUNK)[:, None] // 128) * 128
    filled = np.arange(CHUNK)[None, :] >= ii

    def tri_block(r, tslot, f):
        B = np.zeros((CHUNK, CHUNK), dtype=np.float32)
        tr = r["outtr"][tslot * 128 : (tslot + 1) * 128, :]
        for g in range(8):
            w = CHUNK - 128 * g
            B[g * 128 : (g + 1) * 128, g * 128 :] = _dequant(
                tr[:, TRI_OFF[g] : TRI_OFF[g] + w], f
            )
        return B

    for m in range(NCORES):
        locs = [(m + a) % NCORES for a in LOCAL_OFFS]
        r = results[m]
        fv = r["fv"][0]
        for lp, lq, kind, fs in BLOCKS:
            if kind != "full":
                continue
            b = ORow[fs]
            raw = r["odd"][b * 128 : (b + 1) * 128, :]
            blk = _dequant(
                raw.reshape(128, 8, CHUNK).swapaxes(0, 1).reshape(CHUNK, CHUNK),
                fv[fs],
            )
            P, Q = locs[lp], locs[lq]
            out[P * CHUNK : (P + 1) * CHUNK, Q * CHUNK : (Q + 1) * CHUNK] = blk
            out[Q * CHUNK : (Q + 1) * CHUNK, P * CHUNK : (P + 1) * CHUNK] = blk.T
        B = tri_block(r, 0, fv[0])
        out[m * CHUNK : (m + 1) * CHUNK, m * CHUNK : (m + 1) * CHUNK] = np.where(
            filled, B, B.T
        )
    for m in range(4):
        rA, rB = results[m], results[m + 4]
        BA = tri_block(rA, 1, rA["fv"][0][3])
        BB = tri_block(rB, 1, rB["fv"][0][3])
        X = np.where(filled, BA, BB.T)
        P, Q = m, m + 4
        out[P * CHUNK : (P + 1) * CHUNK, Q * CHUNK : (Q + 1) * CHUNK] = X
        out[Q * CHUNK : (Q + 1) * CHUNK, P * CHUNK : (P + 1) * CHUNK] = X.T
    return out


def kernel(gathered_sequences, W1, b1, w2, b2):
    global LAST_RESULTS
    from concourse import bass_utils
    import ml_dtypes

    bf = ml_dtypes.bfloat16

    if "nc" not in _state:
        _state["nc"] = _build()
    nc = _state["nc"]

    x = np.asarray(gathered_sequences, dtype=np.float32)
    xT = np.ascontiguousarray(x.T).astype(bf)  # [D, N]
    W1b = np.asarray(W1, dtype=np.float32).astype(bf)
    w2c = np.reshape(np.asarray(w2, np.float32), (H, 1)).astype(bf)
    w2b = np.ascontiguousarray(np.repeat(w2c, 128, axis=1))
    b2s = float(np.reshape(np.asarray(b2, np.float32), ()))
    fpack = np.zeros((128, 3), dtype=np.float32)
    fpack[:, 0] = abs(b2s)
    fpack[:, 1] = b2s
    fpack[:H, 2] = np.asarray(b1, np.float32).ravel()

    in_maps = []
    for m in range(NCORES):
        locs = [(m + a) % NCORES for a in LOCAL_OFFS]
        # per chunk: [128, 4096] stripe-major: col = half*2048 + k*512 + j
        xT4 = np.concatenate(
            [
                xT[:, L * CHUNK : (L + 1) * CHUNK]
                .reshape(KCH, 128, 2, STRIPE)
                .transpose(1, 2, 0, 3)
                .reshape(128, KCH * CHUNK)
                for L in locs
            ],
            axis=1,
        )
        in_maps.append(
            {
                "xT4": np.ascontiguousarray(xT4),
                "W1b": W1b,
                "w2b": w2b,
                "fpack": fpack,
            }
        )

    res = bass_utils.run_bass_kernel_spmd(nc, in_maps, core_ids=list(range(NCORES)))
    LAST_RESULTS = res
    return _assemble(res.results)
